# revision 1
# baseline (speedup 1.0000x reference)
"""Trainium2 Bass kernel for nn_DepthCalibration.

Math (per batch b):
  s      = conv1d(pred*g, w, pad=1) + cb                     (smoothed depths)
  e[n,m] = -2*||ray_n - ray_m||^2                            (sigma=0.5 fixed)
  out[n] = clip(sum_m exp(e[n,m]) * s[m], 0.1, 100)

Strategy: one batch per NeuronCore (B=8, 8 cores, fully data parallel).
The exponent is a rank-9 augmented inner product:
  e = 4*r.r' - 2(x^2+y^2+z^2) - 2(x'^2+y'^2+z'^2)
    = matmul(A[:,n], B[:,m])  with
  A = [x, y, z, x^2, y^2, z^2, 1, 1, 1]   (stationary side, f32r)
  B = [4x', 4y', 4z', -2, -2, -2, -2x'^2, -2y'^2, -2z'^2]
so the TensorEngine produces exp-arguments directly into PSUM (f32r runs
at 1 cycle/row vs fp32's 4x-slow path; measured 1.9e-4 matmul accuracy);
ScalarE exp (the 1 elem/lane/cycle floor, ~109us/core) converts to fp16
weights; the weighted row-sum is one fused DVE scalar_tensor_tensor per
128-row block (products computed in fp32 internally, accumulated to a
f32 [128,1] column) against a partition-broadcast copy of s.

Engine budget per core (measured): ACT exp ~121us, DVE mv ~146us,
PE matmuls ~62us, all overlapped; steady-state ~225us/call end to end.
The conv1d smoothing, augmented-matrix construction, and clipping all
run on device; the host only reshapes/pads/transposes inputs for DMA
friendliness (strided 4-byte DRAM reads are descriptor-dominated).
"""

import sys
import os

sys.path.insert(0, "/opt/trn_rl_repo")

import numpy as np

from concourse import bass, mybir
from concourse import bacc
from concourse import tile
from concourse.bass_utils import run_bass_kernel_spmd

B, N = 8, 4096
NB = N // 128          # 32 row blocks of 128
CHUNK = 2048           # ACT chunk (4 PSUM banks)
NCHUNK = N // CHUNK    # 2 chunks per row block
MM = 512               # matmul moving free dim (one PSUM bank of fp32)
MIN_DEPTH, MAX_DEPTH = 0.1, 100.0

F32 = mybir.dt.float32
F32R = mybir.dt.float32r
FP16 = mybir.dt.float16

KAUG = 9               # augmented contraction depth
ALT = True             # alternate PE row groups to hide LDWEIGHTS
WIDE_STT = True        # one [128, N] STT per row block vs per-chunk
WBUFS = 3              # exp-output (W) tile buffers
SCBUFS = 2             # STT scratch-output buffers
SKIP_STT = False       # ablation: drop the DVE weighted-sum
SKIP_EXP = False       # ablation: drop the ACT exp
SKIP_MM = False        # ablation: drop the matmuls
SKIP_PREP_AB = False   # ablation: drop A/B aug build
SKIP_PREP_S = False    # ablation: drop s conv/broadcast chain


def build_program(gw0, gw1, gw2, cb, w_dtype=FP16, repeat=1):
    """Build the single-core program (run SPMD on 8 cores).

    gw0/gw1/gw2: conv taps pre-multiplied by global_scale; cb: conv bias.
    repeat>1 wraps the body in a hardware loop (for timing measurement).
    """
    nc = bacc.Bacc(
        "TRN2",
        target_bir_lowering=False,
        debug=False,
        enable_asserts=False,
        num_devices=8,
    )

    pred_pad = nc.dram_tensor("pred_pad", (N + 2,), F32, kind="ExternalInput").ap()
    rayT = nc.dram_tensor("rayT", (3, N), F32, kind="ExternalInput").ap()
    out = nc.dram_tensor("out", (N,), F32, kind="ExternalOutput").ap()
    s_dram = nc.dram_tensor("s_scratch", (N,), w_dtype, kind="Internal").ap()

    AF = mybir.ActivationFunctionType
    OP = mybir.AluOpType

    from contextlib import ExitStack

    ngrp = 2 if ALT else 1

    with tile.TileContext(nc) as tc, ExitStack() as stk:
        if repeat > 1:
            ET = mybir.EngineType
            stk.enter_context(
                tc.For_i(
                    0,
                    repeat,
                    1,
                    hint_engines=(ET.PE, ET.DVE, ET.Activation, ET.SP, ET.Pool),
                )
            )
        with (
            tc.tile_pool(name="const", bufs=1) as cpool,
            tc.tile_pool(name="w", bufs=WBUFS) as wpool,
            tc.tile_pool(name="ttr", bufs=SCBUFS) as tpool,
            tc.tile_pool(name="psum", bufs=2, space="PSUM") as ppool,
        ):
            # ---------------- aug matrices A (stationary) and B (moving) ----
            # duplicated at base partition 32 so consecutive row blocks use
            # different PE row groups (LDWEIGHTS overlaps in-flight matmuls)
            A = cpool.tile([32 * (ngrp - 1) + KAUG, N], F32R)
            Bm = cpool.tile([32 * (ngrp - 1) + KAUG, N], F32R)
            R = cpool.tile([3, N], F32)      # raw rays (x,y,z rows)
            sqm = cpool.tile([3, N], F32R)   # -2x^2 ...
            r4 = cpool.tile([3, N], F32R)    # 4x ...
            ones3 = nc.inline_tensor(np.ones((3, N), np.float32), "ones3").ap()
            m2s3 = nc.inline_tensor(np.full((3, N), -2.0, np.float32), "m2s3").ap()

            if not SKIP_PREP_AB:
                # A = [r, r^2, -2*1s]; B = [4r', -2*1s, r'^2]
                # squares/copy on ACT (idle at prep); only r4 stays on DVE
                nc.sync.dma_start(R[:], rayT[:, :])
                nc.scalar.activation(A[0:3, :], R[:], AF.Identity)
                nc.scalar.activation(sqm[:], R[:], AF.Square)
                nc.vector.tensor_scalar_mul(r4[:], R[:], 4.0)
                nc.sync.dma_start(A[3:6, :], sqm[:])
                nc.sync.dma_start(A[6:9, :], m2s3.bitcast(F32R))
                nc.sync.dma_start(Bm[0:3, :], r4[:])
                nc.sync.dma_start(Bm[6:9, :], sqm[:])
                nc.sync.dma_start(Bm[3:6, :], m2s3.bitcast(F32R))
                for g in range(1, ngrp):
                    nc.sync.dma_start(A[32 * g : 32 * g + KAUG, :], A[0:KAUG, :])
                    nc.sync.dma_start(Bm[32 * g : 32 * g + KAUG, :], Bm[0:KAUG, :])

            # ---------------- smoothed depths s (vertical layout) -----------
            # V*[p, c] = pred_pad[off + p + 128c];  s[i] for i = p + 128c
            def vload(off):
                t = cpool.tile([128, NB], F32, tag=f"v{off}")
                src = pred_pad[off : off + N].rearrange("(c p) -> p c", p=128)
                nc.sync.dma_start(t[:], src)
                return t

            sv = cpool.tile([128, NB], F32)
            vl, vc, vr = vload(0), vload(1), vload(2)
            if SKIP_PREP_S:
                nc.vector.memset(sv[:], 0.5)
            if not SKIP_PREP_S:
                nc.vector.tensor_scalar_mul(sv[:], vl[:], gw0)
                nc.vector.scalar_tensor_tensor(
                    sv[:], vc[:], gw1, sv[:], OP.mult, OP.add
                )
                nc.vector.scalar_tensor_tensor(
                    sv[:], vr[:], gw2, sv[:], OP.mult, OP.add
                )
                nc.vector.tensor_scalar_add(sv[:], sv[:], cb)
            sv_c = cpool.tile([128, NB], w_dtype)
            nc.vector.tensor_copy(sv_c[:], sv[:])
            # to DRAM (linear: i = p + 128c) and broadcast to 128 partitions
            nc.sync.dma_start(s_dram.rearrange("(c p) -> p c", p=128), sv_c[:])
            s_bc = cpool.tile([128, N], w_dtype)
            for q in range(4):
                sl = slice(q * (N // 4), (q + 1) * (N // 4))
                nc.sync.dma_start(
                    s_bc[:, sl],
                    s_dram[sl].rearrange("(o n) -> o n", o=1).broadcast_to(
                        (128, N // 4)
                    ),
                )

            # ---------------- main loop ------------------------------------
            acc = cpool.tile([128, NB], F32)  # per-row-block accumulators
            accp = cpool.tile([128, NB * NCHUNK], F32)  # per-chunk partials
            for i in range(NB):
                g = 32 * (i % ngrp)
                lhsT = A[g : g + KAUG, i * 128 : (i + 1) * 128]
                if WIDE_STT:
                    wt = wpool.tile([128, N], w_dtype, tag="w")
                    for c in range(NCHUNK):
                        m0 = c * CHUNK
                        pt = ppool.tile([128, CHUNK], F32, tag="ps")
                        if not SKIP_MM:
                            for j in range(CHUNK // MM):
                                nc.tensor.matmul(
                                    pt[:, j * MM : (j + 1) * MM],
                                    lhsT,
                                    Bm[g : g + KAUG, m0 + j * MM : m0 + (j + 1) * MM],
                                )
                        if not SKIP_EXP:
                            nc.scalar.activation(wt[:, m0 : m0 + CHUNK], pt[:], AF.Exp)
                        else:
                            nc.vector.memset(wt[0:1, m0 : m0 + 2], 0.5)
                    if not SKIP_STT:
                        sc = tpool.tile([128, N], w_dtype, tag="sc")
                        nc.vector.scalar_tensor_tensor(
                            sc[:],
                            wt[:],
                            0.0,
                            s_bc[:],
                            OP.bypass,
                            OP.mult,
                            accum_out=acc[:, i : i + 1],
                        )
                    else:
                        nc.vector.memset(acc[:, i : i + 1], 0.5)
                else:
                    acc2 = acc  # per-chunk partials combined below
                    for c in range(NCHUNK):
                        m0 = c * CHUNK
                        pt = ppool.tile([128, CHUNK], F32, tag="ps")
                        for j in range(CHUNK // MM):
                            nc.tensor.matmul(
                                pt[:, j * MM : (j + 1) * MM],
                                lhsT,
                                Bm[g : g + KAUG, m0 + j * MM : m0 + (j + 1) * MM],
                            )
                        wt = wpool.tile([128, CHUNK], w_dtype, tag="w")
                        nc.scalar.activation(wt[:], pt[:], AF.Exp)
                        sc = tpool.tile([128, CHUNK], w_dtype, tag="sc")
                        nc.vector.scalar_tensor_tensor(
                            sc[:],
                            wt[:],
                            0.0,
                            s_bc[:, m0 : m0 + CHUNK],
                            OP.bypass,
                            OP.mult,
                            accum_out=accp[:, i * NCHUNK + c : i * NCHUNK + c + 1],
                        )

            # ---------------- clip + store ---------------------------------
            if not WIDE_STT:
                nc.vector.tensor_add(acc[:], accp[:, 0::NCHUNK], accp[:, 1::NCHUNK])
            res = cpool.tile([128, NB], F32)
            nc.vector.tensor_scalar(
                res[:],
                acc[:],
                MIN_DEPTH,
                MAX_DEPTH,
                OP.max,
                OP.min,
            )
            nc.sync.dma_start(out.rearrange("(i p) -> p i", p=128), res[:])

    nc.compile()
    return nc


_cache = {}


def _get_program(key, gw0, gw1, gw2, cb, w_dtype, repeat=1):
    key = key + (repeat,)
    if key not in _cache:
        _cache[key] = build_program(gw0, gw1, gw2, cb, w_dtype, repeat=repeat)
    return _cache[key]


def kernel(pred_depth, ray_3d, conv_w, conv_b, global_scale, repeat=1):
    pred_depth = np.asarray(pred_depth, np.float32)
    ray_3d = np.asarray(ray_3d, np.float32)
    g = float(np.asarray(global_scale).reshape(-1)[0])
    w = np.asarray(conv_w, np.float32).reshape(-1)
    cb = float(np.asarray(conv_b).reshape(-1)[0])
    gw0, gw1, gw2 = float(w[0] * g), float(w[1] * g), float(w[2] * g)

    nc = _get_program((gw0, gw1, gw2, cb), gw0, gw1, gw2, cb, FP16, repeat=repeat)

    in_maps = []
    for b in range(B):
        pp = np.zeros(N + 2, np.float32)
        pp[1 : N + 1] = pred_depth[b]
        in_maps.append(
            {
                "pred_pad": pp,
                "rayT": np.ascontiguousarray(ray_3d[b].T),
            }
        )
    res = _run_with_retry(nc, in_maps)
    out = np.stack([res.results[b]["out"] for b in range(B)]).astype(np.float32)
    return out


def _run_with_retry(nc, in_maps, tries=3):
    # The shared axon device occasionally reports a transient
    # NRT_EXEC_UNIT_UNRECOVERABLE after a prior process crashed; it
    # recovers within ~20s. Retry rather than failing the whole call.
    import time as _time

    for attempt in range(tries):
        try:
            return run_bass_kernel_spmd(nc, in_maps, core_ids=list(range(B)))
        except Exception:
            if attempt == tries - 1:
                raise
            _time.sleep(25)



# revision 12
# speedup vs baseline: 1.2066x; 1.2066x over previous
"""Trainium2 Bass kernel for nn_DepthCalibration (symmetric-strip version).

Math (per batch b, one batch per NeuronCore, B=8):
  s      = conv1d(pred*g, w, pad=1) + cb          (host-precomputed)
  e[n,m] = -2*||ray_n - ray_m||^2                 (sigma=0.5 fixed)
  out[n] = clip(sum_m exp(e[n,m]) * s[m], 0.1, 100)

The Gaussian weight matrix W = exp(e) is SYMMETRIC, so only the upper
block-triangle is computed: for each 128-row block i the strip
m in [128i, N) is produced (PE rank-9 augmented matmul -> PSUM, ACT exp
-> fp16 SBUF).  That halves both the ACT exp work and the row-sum work
vs the dense kernel.  Each strip is consumed twice:
  row side  out[n in blk i] += sum_{m>=128i} X[n,m] s[m]
            -- DVE (and optionally Pool/GPSIMD) scalar_tensor_tensor
               with accum_out, split by column range to balance engines
  col side  out[m > last row of blk i] += sum_{n in blk i} X[n,m] s[n]
            -- PE matmul, lhsT = s-block (fp16 [128,1]), accumulated
               across strips into a persistent PSUM bank [8,512],
               evacuated chunk-by-chunk to DRAM as strips retire and
               read back in [128, NB] layout for the final combine.
All input prep (augmented A/B matrices, smoothed s) is host numpy; the
device sees a_aug[9,N], b_aug[9,N] (f32->f32r bitcast) and s16[N] fp16.

Engine budget per core (cost-model): ACT exp ~66us, PE ~57us
(28 E-matmul + 27 col-matmul), DVE+Pool row sums ~45us each.
"""

import sys
import os

sys.path.insert(0, "/opt/trn_rl_repo")

import numpy as np

from concourse import bass, mybir
from concourse import bacc
from concourse import tile
from concourse.bass_utils import run_bass_kernel_spmd

B, N = 8, 4096
NB = N // 128          # 32 row blocks of 128
MM = 512               # matmul moving free dim cap
KAUG = 9               # augmented contraction depth
MIN_DEPTH, MAX_DEPTH = 0.1, 100.0

F32 = mybir.dt.float32
F32R = mybir.dt.float32r
FP16 = mybir.dt.float16

ECH_A = 2048           # exp/psum chunk (pool A: 4 banks)
ECH_B = 1536           # exp/psum chunk (pool B: 3 banks)
WBUFS = 3              # exp-output (X) tile buffers
SCBUFS = 2             # STT scratch-output buffers
DVE_SHARE = 1.0        # fraction of row-sum columns on DVE (rest Pool)


def build_program(repeat=1, dve_share=DVE_SHARE):
    nc = bacc.Bacc(
        "TRN2",
        target_bir_lowering=False,
        debug=False,
        enable_asserts=False,
        num_devices=8,
    )

    a_d = nc.dram_tensor("a_aug", (KAUG, N), F32, kind="ExternalInput").ap()
    b_d = nc.dram_tensor("b_aug", (KAUG, N), F32, kind="ExternalInput").ap()
    s_d = nc.dram_tensor("s16", (N,), FP16, kind="ExternalInput").ap()
    sm_d = nc.dram_tensor("smask", (128, 16 * NB), FP16, kind="ExternalInput").ap()
    out = nc.dram_tensor("out", (N,), F32, kind="ExternalOutput").ap()
    col_dram = nc.dram_tensor("col_scratch", (N,), F32, kind="Internal").ap()

    AF = mybir.ActivationFunctionType
    OP = mybir.AluOpType

    from contextlib import ExitStack

    with tile.TileContext(nc) as tc, ExitStack() as stk:
        if repeat > 1:
            ET = mybir.EngineType
            stk.enter_context(
                tc.For_i(
                    0,
                    repeat,
                    1,
                    hint_engines=(ET.PE, ET.DVE, ET.Activation, ET.SP, ET.Pool),
                )
            )
        with (
            tc.tile_pool(name="const", bufs=1) as cpool,
            tc.tile_pool(name="w", bufs=WBUFS) as wpool,
            tc.tile_pool(name="sc", bufs=SCBUFS) as tpool,
            tc.tile_pool(name="psA", bufs=1, space="PSUM") as ppoolA,
            tc.tile_pool(name="psB", bufs=1, space="PSUM") as ppoolB,
            tc.tile_pool(name="colp", bufs=1, space="PSUM") as colpool,
        ):
            # ---------------- constants / inputs ----------------------------
            A = cpool.tile([KAUG, N], F32R)
            Bm = cpool.tile([KAUG, N], F32R)
            nc.sync.dma_start(A[:], a_d.bitcast(F32R)[:, :])
            nc.sync.dma_start(Bm[:], b_d.bitcast(F32R)[:, :])

            sbc = cpool.tile([128, N], FP16)   # s broadcast to 128 partitions
            for q in range(4):
                sl = slice(q * (N // 4), (q + 1) * (N // 4))
                nc.sync.dma_start(
                    sbc[:, sl],
                    s_d[sl].rearrange("(o n) -> o n", o=1).broadcast_to(
                        (128, N // 4)
                    ),
                )
            # masked lhsT windows for the col-side matmuls: smask[:, 16i+8]
            # holds s-block i, zeros elsewhere; slicing [16i+8-c : 16i+16-c]
            # puts s at window column c => output partition c (matmul output
            # base partition must be 0; other partitions accumulate +=0)
            smask = cpool.tile([128, 16 * NB], FP16)
            nc.sync.dma_start(smask[:], sm_d[:, :])

            acc_d = cpool.tile([128, NB], F32)  # DVE row-sum accums
            acc_p = cpool.tile([128, NB], F32)  # Pool row-sum accums
            colsb = cpool.tile([8, 512], F32)   # col accum staging (SBUF)
            colre = cpool.tile([128, NB], F32)  # col-side, re-laid-out
            res = cpool.tile([128, NB], F32)
            colacc = colpool.tile([8, 512], F32)  # col accum, 1 PSUM bank

            nc.vector.memset(acc_p[:], 0.0)
            # reset the whole col accumulator on PE (zero window, start=True):
            # keeps the accumulation chain entirely in PE program order
            nc.tensor.matmul(
                colacc[0:8, 0:512],
                smask[:, 0:8],
                sbc[:, 0:512],
                start=True,
                stop=False,
                skip_group_check=True,
            )

            live = {}
            chunk_ctr = [0]

            def emit_strip(i):
                f = N - 128 * i
                X = wpool.tile([128, N], FP16, tag="w")
                live[i] = (X, f)
                lhsT = A[:, 128 * i : 128 * (i + 1)]
                off = 0
                while off < f:
                    if chunk_ctr[0] % 2 == 0:
                        pool, cap, tg = ppoolA, ECH_A, "ptA"
                    else:
                        pool, cap, tg = ppoolB, ECH_B, "ptB"
                    chunk_ctr[0] += 1
                    L = min(cap, f - off)
                    pt = pool.tile([128, cap], F32, tag=tg)
                    for j0 in range(0, L, MM):
                        ml = min(MM, L - j0)
                        m0 = 128 * i + off + j0
                        nc.tensor.matmul(
                            pt[:, j0 : j0 + ml],
                            lhsT,
                            Bm[:, m0 : m0 + ml],
                            start=True,
                            stop=True,
                        )
                    nc.scalar.activation(X[:, off : off + L], pt[:, 0:L], AF.Exp)
                    off += L
                # row side: weighted row sums against s (free-dim split
                # between DVE and Pool to balance engine time)
                dl = f if f <= 512 else min(f, int(round(f * dve_share / 2.0)) * 2)
                sc = tpool.tile([128, N], FP16, tag="sc")
                nc.vector.scalar_tensor_tensor(
                    sc[:, 0:dl],
                    X[:, 0:dl],
                    0.0,
                    sbc[:, 128 * i : 128 * i + dl],
                    OP.bypass,
                    OP.mult,
                    accum_out=acc_d[:, i : i + 1],
                )
                if dl < f:
                    nc.gpsimd.scalar_tensor_tensor(
                        sc[:, dl:f],
                        X[:, dl:f],
                        0.0,
                        sbc[:, 128 * i + dl : 128 * i + f],
                        OP.bypass,
                        OP.mult,
                        accum_out=acc_p[:, i : i + 1],
                    )

            def emit_col(i):
                # out[m] += sum_{n in blk i} X[n, m] s[n]   for m >= 128(i+1)
                X, f = live.pop(i)
                off = 128 * (i + 1)
                while off < N:
                    c = off // 512
                    end = min(N, 512 * (c + 1))
                    nc.tensor.matmul(
                        colacc[0:8, off - 512 * c : end - 512 * c],
                        smask[:, 16 * i + 8 - c : 16 * i + 16 - c],
                        X[:, off - 128 * i : end - 128 * i],
                        start=False,
                        stop=(i == NB - 2 and end == N),
                        skip_group_check=True,
                    )
                    off = end

            for i in range(NB):
                emit_strip(i)
                if i >= 1:
                    emit_col(i - 1)
            # strip NB-1 has no col side (its strip is just the diagonal blk)

            # ---------------- combine + clip + store ------------------------
            # evacuate col accumulator (m-linear) and read back in the
            # [128, NB] block layout of the row-side accumulators
            nc.vector.tensor_copy(colsb[:], colacc[0:8, :])
            nc.sync.dma_start(
                col_dram.rearrange("(c j) -> c j", c=8), colsb[:]
            )
            nc.sync.dma_start(colre[:], col_dram.rearrange("(i p) -> p i", p=128))
            nc.vector.tensor_add(res[:], acc_d[:], acc_p[:])
            nc.vector.tensor_add(res[:], res[:], colre[:])
            nc.vector.tensor_scalar(
                res[:], res[:], MIN_DEPTH, MAX_DEPTH, OP.max, OP.min
            )
            nc.sync.dma_start(out.rearrange("(i p) -> p i", p=128), res[:])

    nc.compile()
    return nc


def _host_prep(pred_depth, ray_3d, conv_w, conv_b, global_scale):
    pred = np.asarray(pred_depth, np.float32)
    ray = np.asarray(ray_3d, np.float32)
    g = float(np.asarray(global_scale).reshape(-1)[0])
    w = np.asarray(conv_w, np.float32).reshape(-1)
    cb = float(np.asarray(conv_b).reshape(-1)[0])

    x = pred * g
    xp = np.pad(x, ((0, 0), (1, 1)))
    s = (w[0] * xp[:, :-2] + w[1] * xp[:, 1:-1] + w[2] * xp[:, 2:] + cb).astype(
        np.float32
    )

    ones = np.ones(N, np.float32)
    s16 = s.astype(np.float16)
    # smask[p, 16i+8] = s[128i+p]; windows [16i+8-c : 16i+16-c] place the
    # s-block at matmul output partition c
    smask_all = np.zeros((B, 128, 16 * NB), np.float16)
    sblk = s16.reshape(B, NB, 128).transpose(0, 2, 1)  # [B, 128, NB]
    smask_all[:, :, 8::16] = sblk

    in_maps = []
    for b in range(B):
        r = ray[b]
        x0, y0, z0 = r[:, 0], r[:, 1], r[:, 2]
        sq = r * r
        a_aug = np.ascontiguousarray(
            np.stack([x0, y0, z0, sq[:, 0], sq[:, 1], sq[:, 2], ones, ones, ones]),
            np.float32,
        )
        b_aug = np.ascontiguousarray(
            np.stack(
                [
                    4.0 * x0,
                    4.0 * y0,
                    4.0 * z0,
                    -2.0 * ones,
                    -2.0 * ones,
                    -2.0 * ones,
                    -2.0 * sq[:, 0],
                    -2.0 * sq[:, 1],
                    -2.0 * sq[:, 2],
                ]
            ),
            np.float32,
        )
        in_maps.append(
            {
                "a_aug": a_aug,
                "b_aug": b_aug,
                "s16": s16[b],
                "smask": np.ascontiguousarray(smask_all[b]),
            }
        )
    return in_maps


_cache = {}


def _get_program(repeat=1, dve_share=DVE_SHARE):
    key = (repeat, dve_share)
    if key not in _cache:
        _cache[key] = build_program(repeat=repeat, dve_share=dve_share)
    return _cache[key]


def kernel(pred_depth, ray_3d, conv_w, conv_b, global_scale, repeat=1):
    nc = _get_program(repeat=repeat)
    in_maps = _host_prep(pred_depth, ray_3d, conv_w, conv_b, global_scale)
    res = _run_with_retry(nc, in_maps)
    out = np.stack([res.results[b]["out"] for b in range(B)]).astype(np.float32)
    return out


def _run_with_retry(nc, in_maps, tries=3):
    # The shared axon device occasionally reports a transient
    # NRT_EXEC_UNIT_UNRECOVERABLE after a prior process crashed; it
    # recovers within ~20s. Retry rather than failing the whole call.
    import time as _time

    for attempt in range(tries):
        try:
            return run_bass_kernel_spmd(nc, in_maps, core_ids=list(range(B)))
        except Exception:
            if attempt == tries - 1:
                raise
            _time.sleep(25)
